# revision 2
# baseline (speedup 1.0000x reference)
"""NetVLAD-style vq_codebook kernel for 8 Trainium2 NeuronCores.

Reference computation (per full input):
  assn = BN(x @ clusters); softmax over 80 clusters, drop 16 ghosts
  vlad[b,d,k] = sum_n assn[b,n,k] x[b,n,d] - a_sum[b,k]*clusters2[d,k]
  intra-normalize over d, flatten, global L2 normalize -> (B, D*K)

Sharding: data-parallel over batch B (B/8 batches per core). BatchNorm
statistics (sum and sum-of-squares per cluster column) are all-reduced
across the 8 cores (2*80 floats). Everything else is local.

Implementation notes:
 - x is cast to fp16 on load (gpsimd cast-DMA), kept in natural layout
   (token-partition) for the vlad matmul, and transposed on-chip with the
   DMA XBAR transpose into d-partition layout for the assignment matmul.
 - PE matmuls: per token tile 4 accumulating (128x128fp16)@(128x80fp16)
   matmuls for cluster assignment; vlad: per token tile one
   (128x64)@(128x512) matmul accumulating vlad^T = (64k, 512d) per batch,
   plus an N=1 matmul against a ones column for a_sum.
 - BN stats via PE: ones-column stationary matmuls against assn and
   assn^2 accumulate per-column sums in PSUM.
 - softmax without max-subtraction (logits are exactly BN-normalized,
   |logit| <~ 6, exp is safe in fp32).
"""

import sys

for _p in ("/opt/trn_rl_repo", "/root/.axon_site/_ro/trn_rl_repo"):
    if _p not in sys.path:
        sys.path.insert(0, _p)

import numpy as np

import concourse.bacc as bacc
import concourse.mybir as mybir
import concourse.tile as tile
from concourse.bass_utils import run_bass_kernel_spmd

F32 = mybir.dt.float32
F16 = mybir.dt.float16
AX = mybir.AxisListType
OP = mybir.AluOpType
ACTF = mybir.ActivationFunctionType

N_CORES = 8
D = 512
KG = 80          # clusters + ghosts
K = 64           # real clusters
N_SEQ = 2048
TPB = N_SEQ // 128   # token tiles per batch = 16
BN_EPS = 1e-5
L2_EPS = 1e-12


def build(b_loc=4, n_cores=N_CORES, with_collective=True):
    """Build the per-core program. b_loc = batches per core."""
    nt = b_loc * TPB                # token tiles per core
    tok = nt * 128                  # tokens per core
    total_tok = tok * n_cores       # global token count for BN stats

    nc = bacc.Bacc("TRN2", target_bir_lowering=False, debug=False)

    x = nc.declare_dram_parameter("x", [tok, D], F32, isOutput=False)
    cl = nc.declare_dram_parameter("clusters", [D, KG], F32, isOutput=False)
    c2 = nc.declare_dram_parameter("clusters2", [D, K], F32, isOutput=False)
    gam = nc.declare_dram_parameter("bn_gamma", [1, KG], F32, isOutput=False)
    bet = nc.declare_dram_parameter("bn_beta", [1, KG], F32, isOutput=False)
    y = nc.declare_dram_parameter("y", [b_loc, D * K], F32, isOutput=True)

    ident_c = nc.inline_tensor(np.eye(128, dtype=np.float32), name="c_ident")
    ones_col_c = nc.inline_tensor(np.ones((128, 1), np.float32), name="c_ones_col")
    ones_row_c = nc.inline_tensor(np.ones((1, 128), np.float32), name="c_ones_row")

    with tile.TileContext(nc) as tc:
        with (
            tc.tile_pool(name="persist", bufs=1) as persist,
            tc.tile_pool(name="work", bufs=3) as work,
            tc.tile_pool(name="dram", bufs=1, space="DRAM") as dram,
        ):
            # ---- persistent SBUF tensors ----
            xh = persist.tile([128, nt, D + 1], F16, name="xh")
            assn = persist.tile([128, nt, KG], F32, name="assn")
            sm = persist.tile([128, nt, K], F16, name="sm")
            clh = persist.tile([128, 4, KG], F16, name="clh")
            c2n = persist.tile([128, 4, K], F32, name="c2n")
            c2T = persist.tile([64, D], F32, name="c2T")
            ident = persist.tile([128, 128], F32, name="ident")
            ones_col = persist.tile([128, 1], F32, name="ones_col")
            ones_row = persist.tile([1, 128], F32, name="ones_row")
            gamma = persist.tile([1, KG], F32, name="gamma")
            beta = persist.tile([1, KG], F32, name="beta")
            ss = persist.tile([1, 2 * KG], F32, name="ss")
            stats_sb = persist.tile([1, 2 * KG], F32, name="stats_sb")
            stats_g = persist.tile([1, 2 * KG], F32, name="stats_g")
            bcB = persist.tile([128, 2 * KG], F32, name="bcB")
            asum_sb = persist.tile([64, 4], F32, name="asum_sb")

            stats_in = dram.tile([1, 2 * KG], F32, name="stats_in")
            stats_out = dram.tile([1, 2 * KG], F32, name="stats_out")

            # ---- phase 0: constants + x load/cast ----
            nc.sync.dma_start(ident[:], ident_c.ap()[:, :])
            nc.sync.dma_start(ones_col[:], ones_col_c.ap()[:, :])
            nc.sync.dma_start(ones_row[:], ones_row_c.ap()[:, :])
            nc.sync.dma_start(gamma[:], gam[:, :])
            nc.sync.dma_start(beta[:], bet[:, :])
            # clusters -> fp16 chunks (cast dma): chunk c partition p = row 128c+p
            nc.gpsimd.dma_start(
                clh[:], cl.ap().rearrange("(c p) k -> p c k", p=128))
            # clusters2 natural layout; PE-transposed to (64k, 512d) below
            nc.sync.dma_start(
                c2n[:], c2.ap().rearrange("(c p) k -> p c k", p=128))
            nc.vector.memset(xh[:, :, D:D + 1], 1.0)

            # x cast-DMA in groups of 8 token tiles
            xr = x.ap().rearrange("(t p) d -> p t d", p=128)
            for g in range(nt // 8):
                nc.gpsimd.dma_start(
                    xh[:, 8 * g:8 * (g + 1), :D], xr[:, 8 * g:8 * (g + 1), :])

            # ---- phases 0b-2: transposes, assignment matmul, BN stats ----
            with tc.tile_pool(name="ps1", bufs=4, space="PSUM") as ps1:
                for c in range(4):
                    pt = ps1.tile([64, 128], F32, name="pt", tag="c2t", bufs=1)
                    nc.tensor.matmul(pt[:], c2n[:, c, :], ident[:, :],
                                     is_transpose=True, start=True, stop=True)
                    nc.vector.tensor_copy(c2T[:, c * 128:(c + 1) * 128], pt[:])

                pstat = ps1.tile([1, 2 * KG], F32, name="pstat", tag="stat",
                                 bufs=1)
                for t in range(nt):
                    xhTt = work.tile([128, 4, 128], F16, name="xhTt",
                                     tag="xhT", bufs=4)
                    for c in range(4):
                        nc.sync.dma_start(
                            xhTt[:, c, :],
                            xh[:, t, c * 128:(c + 1) * 128], transpose=True)
                    p1 = ps1.tile([128, KG], F32, name="p1", tag="p1")
                    for c in range(4):
                        nc.tensor.matmul(
                            p1[:], xhTt[:, c, :], clh[:, c, :],
                            start=(c == 0), stop=(c == 3),
                            skip_group_check=True)
                    nc.vector.tensor_copy(assn[:, t, :], p1[:])
                    asq = work.tile([128, KG], F32, name="asq", tag="asq")
                    nc.scalar.square(asq[:], assn[:, t, :])
                    nc.tensor.matmul(pstat[:, :KG], ones_col[:], assn[:, t, :],
                                     start=(t == 0), stop=(t == nt - 1),
                                     skip_group_check=True)
                    nc.tensor.matmul(pstat[:, KG:], ones_col[:], asq[:],
                                     start=(t == 0), stop=(t == nt - 1),
                                     skip_group_check=True)

                # ---- phase 2: all-reduce stats ----
                nc.vector.tensor_copy(stats_sb[:], pstat[:])

            nc.sync.dma_start(stats_in[:], stats_sb[:])
            if with_collective:
                nc.gpsimd.collective_compute(
                    "AllReduce", OP.add,
                    replica_groups=[list(range(n_cores))],
                    ins=[stats_in.opt()], outs=[stats_out.opt()])
            else:
                nc.sync.dma_start(stats_out[:], stats_in[:])
            nc.sync.dma_start(stats_g[:], stats_out[:])

            t_mean = work.tile([1, KG], F32, name="t_mean", tag="sv", bufs=6)
            t_var = work.tile([1, KG], F32, name="t_var", tag="sv", bufs=6)
            t_sd = work.tile([1, KG], F32, name="t_sd", tag="sv", bufs=6)
            t_rs = work.tile([1, KG], F32, name="t_rs", tag="sv", bufs=6)
            t_ms = work.tile([1, KG], F32, name="t_ms", tag="sv", bufs=6)
            inv_n = 1.0 / float(total_tok)
            nc.vector.tensor_scalar_mul(t_mean[:], stats_g[:, :KG], inv_n)
            nc.vector.tensor_scalar_mul(t_var[:], stats_g[:, KG:], inv_n)
            nc.vector.tensor_tensor(t_ms[:], t_mean[:], t_mean[:], op=OP.mult)
            nc.vector.tensor_tensor(t_var[:], t_var[:], t_ms[:], op=OP.subtract)
            nc.vector.tensor_scalar_add(t_var[:], t_var[:], BN_EPS)
            nc.scalar.sqrt(t_sd[:], t_var[:])
            nc.vector.reciprocal(t_rs[:], t_sd[:])
            nc.vector.tensor_tensor(ss[:, :KG], t_rs[:], gamma[:], op=OP.mult)
            nc.vector.tensor_tensor(t_ms[:], t_mean[:], ss[:, :KG], op=OP.mult)
            nc.vector.tensor_tensor(ss[:, KG:], beta[:], t_ms[:], op=OP.subtract)

            # ---- phases 3-5 per batch ----
            with (
                tc.tile_pool(name="ps2", bufs=2, space="PSUM") as ps2,
                tc.tile_pool(name="elem", bufs=2) as elem,
                tc.tile_pool(name="vpost", bufs=2) as vpost,
            ):
                pbc = ps2.tile([128, 2 * KG], F32, name="pbc", tag="pbc",
                               bufs=1)
                nc.tensor.matmul(pbc[:], ones_row[:], ss[:], start=True,
                                 stop=True, skip_group_check=True)
                nc.vector.tensor_copy(bcB[:], pbc[:])
                scale_b = bcB[:, :KG].rearrange("p (a k) -> p a k", a=1)
                shift_b = bcB[:, KG:].rearrange("p (a k) -> p a k", a=1)

                pa = ps2.tile([64, 4], F32, name="pa", tag="pa", bufs=1)

                for b in range(b_loc):
                    t0 = b * TPB
                    te = elem.tile([128, TPB, KG], F32, name="te", tag="te")
                    nc.vector.tensor_tensor(
                        te[:], assn[:, t0:t0 + TPB, :],
                        scale_b.to_broadcast([128, TPB, KG]), op=OP.mult)
                    nc.vector.tensor_tensor(
                        te[:], te[:], shift_b.to_broadcast([128, TPB, KG]),
                        op=OP.add)
                    nc.scalar.activation(te[:], te[:], ACTF.Exp)
                    denom = work.tile([128, TPB], F32, name="denom", tag="dn")
                    nc.vector.tensor_reduce(denom[:], te[:], axis=AX.X,
                                            op=OP.add)
                    recip = work.tile([128, TPB], F32, name="recip", tag="rc")
                    nc.vector.reciprocal(recip[:], denom[:])
                    nc.vector.tensor_tensor(
                        sm[:, t0:t0 + TPB, :], te[:, :, :K],
                        recip[:].rearrange("p (t a) -> p t a", a=1)
                        .to_broadcast([128, TPB, K]), op=OP.mult)

                    pv = ps2.tile([64, D], F32, name="pv", tag="pv")
                    for i in range(TPB):
                        t = t0 + i
                        nc.tensor.matmul(pv[:], sm[:, t, :], xh[:, t, :D],
                                         start=(i == 0), stop=(i == TPB - 1),
                                         skip_group_check=True)
                        nc.tensor.matmul(pa[:, b:b + 1], sm[:, t, :],
                                         xh[:, t, D:D + 1],
                                         start=(i == 0), stop=(i == TPB - 1),
                                         skip_group_check=True)

                    # vlad post-processing in (64k, 512d) layout
                    nc.vector.tensor_copy(asum_sb[:, b:b + 1], pa[:, b:b + 1])
                    av = vpost.tile([64, D], F32, name="av", tag="av")
                    nc.vector.tensor_scalar(
                        av[:], c2T[:], asum_sb[:, b:b + 1], None, op0=OP.mult)
                    v = vpost.tile([64, D], F32, name="v", tag="v")
                    nc.vector.tensor_tensor(v[:], pv[:], av[:], op=OP.subtract)
                    tmp = vpost.tile([64, D], F32, name="tmp", tag="tmp")
                    nc.vector.tensor_tensor(tmp[:], v[:], v[:], op=OP.mult)
                    nrm2 = work.tile([64, 1], F32, name="nrm2", tag="w1")
                    nc.vector.tensor_reduce(nrm2[:], tmp[:], axis=AX.X,
                                            op=OP.add)
                    snorm = work.tile([64, 1], F32, name="snorm", tag="w1")
                    nc.scalar.sqrt(snorm[:], nrm2[:])
                    nc.vector.tensor_scalar_max(snorm[:], snorm[:], L2_EPS)
                    rn = work.tile([64, 1], F32, name="rn", tag="w1")
                    nc.vector.reciprocal(rn[:], snorm[:])
                    # global norm: g2 = sum_k (snorm*rn)^2 -> 1/max(sqrt,eps)
                    t1 = work.tile([64, 1], F32, name="t1", tag="w1")
                    nc.vector.tensor_tensor(t1[:], snorm[:], rn[:], op=OP.mult)
                    nc.vector.tensor_tensor(t1[:], t1[:], t1[:], op=OP.mult)
                    pg = ps2.tile([1, 1], F32, name="pg", tag="tiny")
                    nc.tensor.matmul(pg[:], t1[:], ones_col[:64, :],
                                     start=True, stop=True,
                                     skip_group_check=True)
                    g2 = work.tile([1, 1], F32, name="g2", tag="w1")
                    nc.scalar.sqrt(g2[:], pg[:])
                    nc.vector.tensor_scalar_max(g2[:], g2[:], L2_EPS)
                    gr = work.tile([1, 1], F32, name="gr", tag="w1")
                    nc.vector.reciprocal(gr[:], g2[:])
                    pgb = ps2.tile([64, 1], F32, name="pgb", tag="tiny")
                    nc.tensor.matmul(pgb[:], ones_row[:, :64], gr[:],
                                     start=True, stop=True,
                                     skip_group_check=True)
                    nc.vector.tensor_tensor(rn[:], rn[:], pgb[:], op=OP.mult)
                    vf = vpost.tile([64, D], F32, name="vf", tag="vf")
                    nc.vector.tensor_scalar(vf[:], v[:], rn[:], None,
                                            op0=OP.mult)

                    yb = y[b, :].rearrange("(d k) -> d k", k=K)
                    for c in range(4):
                        ptb = ps2.tile([128, K], F32, name="ptb", tag="ptb")
                        nc.tensor.transpose(ptb[:],
                                            vf[:, c * 128:(c + 1) * 128],
                                            ident[:64, :64])
                        vo = work.tile([128, K], F32, name="vo", tag="vo")
                        nc.vector.tensor_copy(vo[:], ptb[:])
                        nc.sync.dma_start(yb[c * 128:(c + 1) * 128, :], vo[:])
    nc.compile()
    return nc


_CACHE = {}


def _get(b_loc, n_cores, with_collective):
    key = (b_loc, n_cores, with_collective)
    if key not in _CACHE:
        _CACHE[key] = build(b_loc, n_cores, with_collective)
    return _CACHE[key]


def make_in_maps(x, clusters, clusters2, bn_gamma, bn_beta, n_cores=N_CORES):
    B = x.shape[0]
    b_loc = B // n_cores
    shared = {
        "clusters": np.ascontiguousarray(clusters, np.float32),
        "clusters2": np.ascontiguousarray(
            np.asarray(clusters2).reshape(D, K), np.float32),
        "bn_gamma": np.ascontiguousarray(
            np.asarray(bn_gamma).reshape(1, KG), np.float32),
        "bn_beta": np.ascontiguousarray(
            np.asarray(bn_beta).reshape(1, KG), np.float32),
    }
    in_maps = []
    for i in range(n_cores):
        m = dict(shared)
        m["x"] = np.ascontiguousarray(
            np.asarray(x[i * b_loc:(i + 1) * b_loc]).reshape(
                b_loc * N_SEQ, D), np.float32)
        in_maps.append(m)
    return in_maps


def kernel(x, clusters, clusters2, bn_gamma, bn_beta):
    B, N, Dd = x.shape
    assert (N, Dd) == (N_SEQ, D) and B % N_CORES == 0
    b_loc = B // N_CORES
    nc = _get(b_loc, N_CORES, True)
    in_maps = make_in_maps(x, clusters, clusters2, bn_gamma, bn_beta)
    res = run_bass_kernel_spmd(nc, in_maps, core_ids=list(range(N_CORES)))
    out = np.concatenate([res.results[i]["y"] for i in range(N_CORES)], axis=0)
    return out
